# revision 2
# baseline (speedup 1.0000x reference)
"""NeuMF (embedding lookup + tiny MLP) on 8 Trainium2 NeuronCores.

Strategy (data-parallel, per the sharding hint's "replicate tables, shard ids"):
- Host: concatenate gmf+mlp halves of each table into one combined table
  cucm[(NU+NM), 72] (user row r = [gmf_user[r] | mlp_user[r]], movie rows
  offset by NU).  Shard the 16384 batch ids 8 ways; each core gathers its
  2048 user rows + 2048 movie rows with 32 indirect DMAs (128 rows each,
  one row per SBUF partition), then computes the whole model on-chip:
    prod  = gmf_u * gmf_m                  (DVE)
    transposes via PE (batch -> free axis)
    h1    = relu(W1^T mlp_in + b1)         (block-diag matmuls, 8 t-blocks/chunk)
    h2    = relu(W2^T h1 + b2)
    logit = Wf_gmf^T prodT + Wf_mlp^T h2   (PSUM-accumulated matmuls)
    out   = 4*sigmoid(logit + bf) + 1      (ACT)
- Gather/unshard on host (pure layout permutation, no embedding math).
"""
import sys
import types
import functools

import numpy as np

# ---------------- problem constants (hardcoded per contract) ----------------
NU = 1_000_000
NM = 100_000
E = 64            # gmf embed dim
MD = 8            # mlp half dim
CW = E + MD       # combined row width (72 f32)
B = 16384
NCORES = 8
SHARD = B // NCORES   # 2048
P = 128
T = SHARD // P        # 16 t-blocks per core
GC = T // 2           # 8 gmf transpose chunks ([128,128] = 2 t-blocks)
MC = T // 8           # 2 mlp transpose chunks ([128,128] = 8 t-blocks)

TRACE = False          # test.py flips this for neuron-profile timing
LAST_EXEC_NS = None


def _install_ntff_hook():
    """bass_utils' trace path imports antenv.axon_hooks (absent here); shim it."""
    if "antenv.axon_hooks" in sys.modules:
        return
    try:
        import antenv  # noqa: F401
        mod = types.ModuleType("antenv.axon_hooks")
        mod._hook = None
        mod.set_axon_ntff_profile_hook = lambda h: setattr(mod, "_hook", h)
        mod.get_axon_ntff_profile_hook = lambda: mod._hook
        sys.modules["antenv.axon_hooks"] = mod
        from trn_agent_boot.trn_boot import _ntff_profile_via_ctypes
        mod.set_axon_ntff_profile_hook(
            _ntff_profile_via_ctypes('/opt/axon/libaxon_pjrt.so'))
    except Exception:
        pass


@functools.lru_cache(maxsize=1)
def _build_program():
    import concourse.bacc as bacc
    import concourse.bass as bass
    import concourse.tile as tile
    from concourse import mybir

    f32 = mybir.dt.float32
    i32 = mybir.dt.int32

    nc = bacc.Bacc("TRN2", target_bir_lowering=False, debug=False,
                   enable_asserts=False, num_devices=NCORES)

    # ids: [128, 32] int32; col 2t = user idx of t-block t, col 2t+1 = movie idx + NU
    ids_d = nc.dram_tensor("ids", (P, 2 * T), i32, kind="ExternalInput")
    tab_d = nc.dram_tensor("tab", (NU + NM, CW), f32, kind="ExternalInput")
    # c128: [128, 128 identity | 128 gw | 64 W1bd] = [128, 320]
    c128_d = nc.dram_tensor("c128", (P, 320), f32, kind="ExternalInput")
    # c64: [64, 32 W2bd | 32 wf4s (rows 0:32) | 1 b1r | 1 b2r | 1 bfr] = [64, 67]
    c64_d = nc.dram_tensor("c64", (64, 67), f32, kind="ExternalInput")
    out_d = nc.dram_tensor("out", (SHARD,), f32, kind="ExternalOutput")

    with tile.TileContext(nc) as tc:
        with (
            tc.tile_pool(name="const", bufs=1) as cpool,
            tc.tile_pool(name="gat", bufs=1) as gpool,
            tc.tile_pool(name="work", bufs=2) as wpool,
            tc.tile_pool(name="ps_t", bufs=2, space="PSUM") as pt_pool,
            tc.tile_pool(name="ps_m", bufs=2, space="PSUM") as pm_pool,
            tc.tile_pool(name="ps_l", bufs=1, space="PSUM") as pl_pool,
        ):
            c128 = cpool.tile([P, 320], f32)
            nc.sync.dma_start(out=c128[:], in_=c128_d[:])
            c64 = cpool.tile([64, 67], f32)
            nc.sync.dma_start(out=c64[:], in_=c64_d[:])
            ids = cpool.tile([P, 2 * T], i32)
            nc.sync.dma_start(out=ids[:], in_=ids_d[:])

            identity = c128[:, 0:128]
            gw = c128[:, 128:256]         # [128, 8 chunks x 16 cols]
            w1bd = c128[:, 256:320]       # [128, 64]
            w2bd = c64[:, 0:32]           # [64, 32]
            wf4s = c64[0:32, 32:64]       # [32, 2 chunks x 16]
            b1r = c64[:, 64:65]           # [64, 1]
            b2r = c64[0:32, 65:66]        # [32, 1]
            bfr = c64[0:16, 66:67]        # [16, 1]

            # ---- gather: 32 indirect DMAs, one [128, 72] row-block each ----
            g = gpool.tile([P, 2 * T * CW], f32)   # [128, 32, 72] flat
            for c in range(2 * T):
                nc.gpsimd.indirect_dma_start(
                    out=g[:, c * CW:(c + 1) * CW],
                    out_offset=None,
                    in_=tab_d[:],
                    in_offset=bass.IndirectOffsetOnAxis(ap=ids[:, c:c + 1], axis=0),
                )

            g3 = g[:].rearrange("p (c w) -> p c w", w=CW)   # [128, 32, 72]
            gu = g3[:, 0::2, :]    # [128, 16, 72] user rows
            gm = g3[:, 1::2, :]    # [128, 16, 72] movie rows

            # ---- gmf elementwise product ----
            prod = wpool.tile([P, T * E], f32, bufs=1)      # [128, 16, 64]
            nc.vector.tensor_mul(
                out=prod[:].rearrange("p (t e) -> p t e", e=E),
                in0=gu[:, :, 0:E], in1=gm[:, :, 0:E])

            # ---- mlp input compaction: [128, 16 t, 16 k] ----
            mlpc = wpool.tile([P, T * 16], f32, bufs=1)
            m3 = mlpc[:].rearrange("p (t k) -> p t k", k=16)
            nc.vector.tensor_copy(out=m3[:, :, 0:MD], in_=gu[:, :, E:CW])
            nc.vector.tensor_copy(out=m3[:, :, MD:16], in_=gm[:, :, E:CW])

            logit = pl_pool.tile([T, P], f32, space="PSUM")
            n_acc = GC + MC
            acc = 0

            # ---- gmf dot: transpose [128,128] chunks, matmul against gw ----
            for r in range(GC):
                pT_ps = pt_pool.tile([P, P], f32, space="PSUM", name="pT_ps", tag="tr")
                nc.tensor.transpose(
                    out=pT_ps[:], in_=prod[:, r * P:(r + 1) * P], identity=identity)
                pT = wpool.tile([P, P], f32, name="pT")
                nc.vector.tensor_copy(out=pT[:], in_=pT_ps[:])
                nc.tensor.matmul(
                    out=logit[:], lhsT=gw[:, r * 16:(r + 1) * 16], rhs=pT[:],
                    start=(acc == 0), stop=(acc == n_acc - 1))
                acc += 1

            # ---- mlp chain per chunk of 8 t-blocks ----
            from concourse.mybir import ActivationFunctionType as AFT
            for c in range(MC):
                mT_ps = pt_pool.tile([P, P], f32, space="PSUM", name="mT_ps", tag="tr")
                nc.tensor.transpose(
                    out=mT_ps[:], in_=mlpc[:, c * P:(c + 1) * P], identity=identity)
                mT = wpool.tile([P, P], f32, name="mT")
                nc.vector.tensor_copy(out=mT[:], in_=mT_ps[:])
                h1_ps = pm_pool.tile([64, P], f32, space="PSUM", name="h1_ps", tag="mm")
                nc.tensor.matmul(out=h1_ps[:], lhsT=w1bd, rhs=mT[:],
                                 start=True, stop=True)
                h1 = wpool.tile([64, P], f32, name="h1")
                nc.scalar.activation(out=h1[:], in_=h1_ps[:], func=AFT.Relu,
                                     bias=b1r)
                h2_ps = pm_pool.tile([64, P], f32, space="PSUM", name="h2_ps", tag="mm")
                nc.tensor.matmul(out=h2_ps[0:32, :], lhsT=w2bd, rhs=h1[:],
                                 start=True, stop=True)
                h2 = wpool.tile([32, P], f32, name="h2")
                nc.scalar.activation(out=h2[:], in_=h2_ps[0:32, :], func=AFT.Relu,
                                     bias=b2r)
                nc.tensor.matmul(
                    out=logit[:], lhsT=wf4s[:, c * 16:(c + 1) * 16], rhs=h2[:],
                    start=(acc == 0), stop=(acc == n_acc - 1))
                acc += 1

            # ---- sigmoid + affine + store ----
            sg = wpool.tile([T, P], f32, bufs=1)
            nc.scalar.activation(out=sg[:], in_=logit[:], func=AFT.Sigmoid,
                                 bias=bfr)
            o = wpool.tile([T, P], f32, bufs=1)
            nc.scalar.activation(out=o[:], in_=sg[:], func=AFT.Copy,
                                 scale=4.0, bias=1.0)
            nc.sync.dma_start(
                out=out_d[:].rearrange("(t p) -> t p", p=P), in_=o[:])

    nc.compile()
    return nc


def _host_prep(user_ids, movie_ids, gmf_user_emb, gmf_movie_emb,
               mlp_user_emb, mlp_movie_emb, W1, b1, W2, b2, Wf, bf):
    """Build the combined table, per-core id layouts, and constant tensors."""
    uid = np.asarray(user_ids).astype(np.int32)
    mid = np.asarray(movie_ids).astype(np.int32)
    tab = np.empty((NU + NM, CW), np.float32)
    tab[:NU, :E] = gmf_user_emb
    tab[:NU, E:] = mlp_user_emb
    tab[NU:, :E] = gmf_movie_emb
    tab[NU:, E:] = mlp_movie_emb

    W1 = np.asarray(W1, np.float32)
    W2 = np.asarray(W2, np.float32)
    Wf = np.asarray(Wf, np.float32)
    b1 = np.asarray(b1, np.float32)
    b2 = np.asarray(b2, np.float32)
    bfv = float(np.asarray(bf).reshape(-1)[0])

    identity = np.eye(P, dtype=np.float32)
    # gw: per gmf chunk r (2 t-blocks), lhsT [128, 16]:
    #   rows 0:64  (parity 0, e) -> col 2r
    #   rows 64:128 (parity 1, e) -> col 2r+1
    gw = np.zeros((P, GC * 16), np.float32)
    for r in range(GC):
        gw[0:E, r * 16 + 2 * r] = Wf[0:E, 0]
        gw[E:2 * E, r * 16 + 2 * r + 1] = Wf[0:E, 0]
    # W1 blockdiag over 8 t_l blocks: [128=(t_l,k), 64=(t_l,j)]
    w1bd = np.zeros((P, 64), np.float32)
    for tl in range(8):
        w1bd[tl * 16:(tl + 1) * 16, tl * 8:(tl + 1) * 8] = W1
    # W2 blockdiag: [64=(t_l,j), 32=(t_l,l)]
    w2bd = np.zeros((64, 32), np.float32)
    for tl in range(8):
        w2bd[tl * 8:(tl + 1) * 8, tl * 4:(tl + 1) * 4] = W2
    # wf4 stage per mlp chunk c: lhsT [32=(t_l,l), 16=t]: col 8c+t_l gets Wf[64+l]
    wf4s = np.zeros((32, MC * 16), np.float32)
    for c in range(MC):
        for tl in range(8):
            wf4s[tl * 4:(tl + 1) * 4, c * 16 + 8 * c + tl] = Wf[E:E + 4, 0]
    b1r = np.tile(b1, 8).reshape(64, 1)
    b2r = np.tile(b2, 8).reshape(32, 1)
    bfr = np.full((16, 1), bfv, np.float32)

    c128 = np.zeros((P, 320), np.float32)
    c128[:, 0:128] = identity
    c128[:, 128:256] = gw
    c128[:, 256:320] = w1bd
    c64 = np.zeros((64, 67), np.float32)
    c64[:, 0:32] = w2bd
    c64[0:32, 32:64] = wf4s
    c64[:, 64:65] = b1r
    c64[0:32, 65:66] = b2r
    c64[0:16, 66:67] = bfr

    in_maps = []
    for c in range(NCORES):
        us = uid[c * SHARD:(c + 1) * SHARD]
        ms = mid[c * SHARD:(c + 1) * SHARD] + NU
        # batch b = t*128 + p maps to ids[p, 2t] / ids[p, 2t+1]
        ids = np.empty((P, 2 * T), np.int32)
        ids[:, 0::2] = us.reshape(T, P).T
        ids[:, 1::2] = ms.reshape(T, P).T
        in_maps.append({"ids": ids, "tab": tab, "c128": c128, "c64": c64})
    return in_maps


def kernel(**inputs) -> np.ndarray:
    global LAST_EXEC_NS
    _install_ntff_hook()
    from concourse.bass_utils import run_bass_kernel_spmd

    nc = _build_program()
    in_maps = _host_prep(**inputs)
    res = run_bass_kernel_spmd(nc, in_maps, list(range(NCORES)), trace=TRACE)
    LAST_EXEC_NS = res.exec_time_ns
    out = np.concatenate([res.results[c]["out"] for c in range(NCORES)])
    return out.astype(np.float32)


# revision 3
# speedup vs baseline: 1.1618x; 1.1618x over previous
"""NeuMF (embedding lookup + tiny MLP) on 8 Trainium2 NeuronCores.

Strategy (data-parallel, per the sharding hint's "replicate tables, shard ids"):
- Host: concatenate gmf+mlp halves of each table into one combined table
  cucm[(NU+NM), 72] (user row r = [gmf_user[r] | mlp_user[r]], movie rows
  offset by NU).  Shard the 16384 batch ids 8 ways; each core gathers its
  2048 user rows + 2048 movie rows with 32 indirect DMAs (128 rows each,
  one row per SBUF partition), then computes the whole model on-chip:
    prod  = gmf_u * gmf_m                  (DVE)
    transposes via PE (batch -> free axis)
    h1    = relu(W1^T mlp_in + b1)         (block-diag matmuls, 8 t-blocks/chunk)
    h2    = relu(W2^T h1 + b2)
    logit = Wf_gmf^T prodT + Wf_mlp^T h2   (PSUM-accumulated matmuls)
    out   = 4*sigmoid(logit + bf) + 1      (ACT)
- Gather/unshard on host (pure layout permutation, no embedding math).
"""
import sys
import types
import functools

import numpy as np

# ---------------- problem constants (hardcoded per contract) ----------------
NU = 1_000_000
NM = 100_000
E = 64            # gmf embed dim
MD = 8            # mlp half dim
CW = E + MD       # combined row width (72 f32)
B = 16384
NCORES = 8
SHARD = B // NCORES   # 2048
P = 128
T = SHARD // P        # 16 t-blocks per core
GC = T // 2           # 8 gmf transpose chunks ([128,128] = 2 t-blocks)
MC = T // 8           # 2 mlp transpose chunks ([128,128] = 8 t-blocks)

TRACE = False          # test.py flips this for neuron-profile timing
LAST_EXEC_NS = None


def _install_ntff_hook():
    """bass_utils' trace path imports antenv.axon_hooks (absent here); shim it."""
    if "antenv.axon_hooks" in sys.modules:
        return
    try:
        import antenv  # noqa: F401
        mod = types.ModuleType("antenv.axon_hooks")
        mod._hook = None
        mod.set_axon_ntff_profile_hook = lambda h: setattr(mod, "_hook", h)
        mod.get_axon_ntff_profile_hook = lambda: mod._hook
        sys.modules["antenv.axon_hooks"] = mod
        from trn_agent_boot.trn_boot import _ntff_profile_via_ctypes
        mod.set_axon_ntff_profile_hook(
            _ntff_profile_via_ctypes('/opt/axon/libaxon_pjrt.so'))
    except Exception:
        pass


@functools.lru_cache(maxsize=1)
def _build_program():
    import concourse.bacc as bacc
    import concourse.bass as bass
    import concourse.tile as tile
    from concourse import mybir

    f32 = mybir.dt.float32
    i32 = mybir.dt.int32

    nc = bacc.Bacc("TRN2", target_bir_lowering=False, debug=False,
                   enable_asserts=False, num_devices=NCORES)

    # ids: [128, 32] int32; col 2t = user idx of t-block t, col 2t+1 = movie idx + NU
    ids_d = nc.dram_tensor("ids", (P, 2 * T), i32, kind="ExternalInput")
    tab_d = nc.dram_tensor("tab", (NU + NM, CW), f32, kind="ExternalInput")
    # c128: [128, 128 identity | 128 gw | 64 W1bd] = [128, 320]
    c128_d = nc.dram_tensor("c128", (P, 320), f32, kind="ExternalInput")
    # c64: [64, 32 W2bd | 32 wf4s (rows 0:32) | 1 b1r | 1 b2r | 1 bfr] = [64, 67]
    c64_d = nc.dram_tensor("c64", (64, 67), f32, kind="ExternalInput")
    out_d = nc.dram_tensor("out", (SHARD,), f32, kind="ExternalOutput")

    with tile.TileContext(nc) as tc:
        with (
            tc.tile_pool(name="const", bufs=1) as cpool,
            tc.tile_pool(name="gat", bufs=1) as gpool,
            tc.tile_pool(name="work", bufs=2) as wpool,
            tc.tile_pool(name="ps_t", bufs=2, space="PSUM") as pt_pool,
            tc.tile_pool(name="ps_m", bufs=2, space="PSUM") as pm_pool,
            tc.tile_pool(name="ps_l", bufs=1, space="PSUM") as pl_pool,
        ):
            c128 = cpool.tile([P, 320], f32)
            nc.sync.dma_start(out=c128[:], in_=c128_d[:])
            c64 = cpool.tile([64, 67], f32)
            nc.sync.dma_start(out=c64[:], in_=c64_d[:])
            ids = cpool.tile([P, 2 * T], i32)
            nc.sync.dma_start(out=ids[:], in_=ids_d[:])

            identity = c128[:, 0:128]
            gw = c128[:, 128:256]         # [128, 8 chunks x 16 cols]
            w1bd = c128[:, 256:320]       # [128, 64]
            w2bd = c64[:, 0:32]           # [64, 32]
            wf4s = c64[0:32, 32:64]       # [32, 2 chunks x 16]
            b1r = c64[:, 64:65]           # [64, 1]
            b2r = c64[0:32, 65:66]        # [32, 1]
            bfr = c64[0:16, 66:67]        # [16, 1]

            # ---- gather: 32 indirect DMAs, one [128, 72] row-block each ----
            g = gpool.tile([P, 2 * T * CW], f32)   # [128, 32, 72] flat
            for c in range(2 * T):
                nc.gpsimd.indirect_dma_start(
                    out=g[:, c * CW:(c + 1) * CW],
                    out_offset=None,
                    in_=tab_d[:],
                    in_offset=bass.IndirectOffsetOnAxis(ap=ids[:, c:c + 1], axis=0),
                )

            g3 = g[:].rearrange("p (c w) -> p c w", w=CW)   # [128, 32, 72]
            gu = g3[:, 0::2, :]    # [128, 16, 72] user rows
            gm = g3[:, 1::2, :]    # [128, 16, 72] movie rows

            from concourse.mybir import ActivationFunctionType as AFT
            # warm the sigmoid ACT LUT during the gathers, off the critical path
            warm = wpool.tile([1, 1], f32, bufs=1)
            nc.scalar.activation(out=warm[:], in_=c64[0:1, 0:1], func=AFT.Sigmoid)

            prod = wpool.tile([P, T * E], f32, bufs=1)      # [128, 16, 64]
            mlpc = wpool.tile([P, T * 16], f32, bufs=1)     # [128, 16, 16]
            m3 = mlpc[:].rearrange("p (t k) -> p t k", k=16)
            logit = pl_pool.tile([T, P], f32, space="PSUM")
            n_acc = GC + MC
            acc = 0

            def mlp_chain(c, acc, n_acc):
                mT_ps = pt_pool.tile([P, P], f32, space="PSUM", name="mT_ps", tag="tr")
                nc.tensor.transpose(
                    out=mT_ps[:], in_=mlpc[:, c * P:(c + 1) * P], identity=identity)
                mT = wpool.tile([P, P], f32, name="mT")
                nc.vector.tensor_copy(out=mT[:], in_=mT_ps[:])
                h1_ps = pm_pool.tile([64, P], f32, space="PSUM", name="h1_ps", tag="mm")
                nc.tensor.matmul(out=h1_ps[:], lhsT=w1bd, rhs=mT[:],
                                 start=True, stop=True)
                h1 = wpool.tile([64, P], f32, name="h1")
                nc.scalar.activation(out=h1[:], in_=h1_ps[:], func=AFT.Relu,
                                     bias=b1r)
                h2_ps = pm_pool.tile([64, P], f32, space="PSUM", name="h2_ps", tag="mm")
                nc.tensor.matmul(out=h2_ps[0:32, :], lhsT=w2bd, rhs=h1[:],
                                 start=True, stop=True)
                h2 = wpool.tile([32, P], f32, name="h2")
                nc.scalar.activation(out=h2[:], in_=h2_ps[0:32, :], func=AFT.Relu,
                                     bias=b2r)
                nc.tensor.matmul(
                    out=logit[:], lhsT=wf4s[:, c * 16:(c + 1) * 16], rhs=h2[:],
                    start=(acc == 0), stop=(acc == n_acc - 1))

            # ---- per 2-t-block chunk: prod/mlp prep + transpose + matmul,
            #      each starts as soon as its 4 gathers have landed ----
            for r in range(GC):
                nc.vector.tensor_mul(
                    out=prod[:, r * P:(r + 1) * P].rearrange(
                        "p (t e) -> p t e", e=E),
                    in0=gu[:, 2 * r:2 * r + 2, 0:E],
                    in1=gm[:, 2 * r:2 * r + 2, 0:E])
                nc.vector.tensor_copy(out=m3[:, 2 * r:2 * r + 2, 0:MD],
                                      in_=gu[:, 2 * r:2 * r + 2, E:CW])
                nc.vector.tensor_copy(out=m3[:, 2 * r:2 * r + 2, MD:16],
                                      in_=gm[:, 2 * r:2 * r + 2, E:CW])
                pT_ps = pt_pool.tile([P, P], f32, space="PSUM", name="pT_ps", tag="tr")
                nc.tensor.transpose(
                    out=pT_ps[:], in_=prod[:, r * P:(r + 1) * P], identity=identity)
                pT = wpool.tile([P, P], f32, name="pT")
                nc.vector.tensor_copy(out=pT[:], in_=pT_ps[:])
                nc.tensor.matmul(
                    out=logit[:], lhsT=gw[:, r * 16:(r + 1) * 16], rhs=pT[:],
                    start=(acc == 0), stop=False)
                acc += 1
                if r == 3 or r == GC - 1:
                    mlp_chain((0 if r == 3 else 1), acc, n_acc)
                    acc += 1

            # ---- sigmoid + affine + store ----
            sg = wpool.tile([T, P], f32, bufs=1)
            nc.scalar.activation(out=sg[:], in_=logit[:], func=AFT.Sigmoid,
                                 bias=bfr)
            o = wpool.tile([T, P], f32, bufs=1)
            nc.scalar.activation(out=o[:], in_=sg[:], func=AFT.Copy,
                                 scale=4.0, bias=1.0)
            nc.sync.dma_start(
                out=out_d[:].rearrange("(t p) -> t p", p=P), in_=o[:])

    nc.compile()
    return nc


def _host_prep(user_ids, movie_ids, gmf_user_emb, gmf_movie_emb,
               mlp_user_emb, mlp_movie_emb, W1, b1, W2, b2, Wf, bf):
    """Build the combined table, per-core id layouts, and constant tensors."""
    uid = np.asarray(user_ids).astype(np.int32)
    mid = np.asarray(movie_ids).astype(np.int32)
    tab = np.empty((NU + NM, CW), np.float32)
    tab[:NU, :E] = gmf_user_emb
    tab[:NU, E:] = mlp_user_emb
    tab[NU:, :E] = gmf_movie_emb
    tab[NU:, E:] = mlp_movie_emb

    W1 = np.asarray(W1, np.float32)
    W2 = np.asarray(W2, np.float32)
    Wf = np.asarray(Wf, np.float32)
    b1 = np.asarray(b1, np.float32)
    b2 = np.asarray(b2, np.float32)
    bfv = float(np.asarray(bf).reshape(-1)[0])

    identity = np.eye(P, dtype=np.float32)
    # gw: per gmf chunk r (2 t-blocks), lhsT [128, 16]:
    #   rows 0:64  (parity 0, e) -> col 2r
    #   rows 64:128 (parity 1, e) -> col 2r+1
    gw = np.zeros((P, GC * 16), np.float32)
    for r in range(GC):
        gw[0:E, r * 16 + 2 * r] = Wf[0:E, 0]
        gw[E:2 * E, r * 16 + 2 * r + 1] = Wf[0:E, 0]
    # W1 blockdiag over 8 t_l blocks: [128=(t_l,k), 64=(t_l,j)]
    w1bd = np.zeros((P, 64), np.float32)
    for tl in range(8):
        w1bd[tl * 16:(tl + 1) * 16, tl * 8:(tl + 1) * 8] = W1
    # W2 blockdiag: [64=(t_l,j), 32=(t_l,l)]
    w2bd = np.zeros((64, 32), np.float32)
    for tl in range(8):
        w2bd[tl * 8:(tl + 1) * 8, tl * 4:(tl + 1) * 4] = W2
    # wf4 stage per mlp chunk c: lhsT [32=(t_l,l), 16=t]: col 8c+t_l gets Wf[64+l]
    wf4s = np.zeros((32, MC * 16), np.float32)
    for c in range(MC):
        for tl in range(8):
            wf4s[tl * 4:(tl + 1) * 4, c * 16 + 8 * c + tl] = Wf[E:E + 4, 0]
    b1r = np.tile(b1, 8).reshape(64, 1)
    b2r = np.tile(b2, 8).reshape(32, 1)
    bfr = np.full((16, 1), bfv, np.float32)

    c128 = np.zeros((P, 320), np.float32)
    c128[:, 0:128] = identity
    c128[:, 128:256] = gw
    c128[:, 256:320] = w1bd
    c64 = np.zeros((64, 67), np.float32)
    c64[:, 0:32] = w2bd
    c64[0:32, 32:64] = wf4s
    c64[:, 64:65] = b1r
    c64[0:32, 65:66] = b2r
    c64[0:16, 66:67] = bfr

    in_maps = []
    for c in range(NCORES):
        us = uid[c * SHARD:(c + 1) * SHARD]
        ms = mid[c * SHARD:(c + 1) * SHARD] + NU
        # batch b = t*128 + p maps to ids[p, 2t] / ids[p, 2t+1]
        ids = np.empty((P, 2 * T), np.int32)
        ids[:, 0::2] = us.reshape(T, P).T
        ids[:, 1::2] = ms.reshape(T, P).T
        in_maps.append({"ids": ids, "tab": tab, "c128": c128, "c64": c64})
    return in_maps


def kernel(**inputs) -> np.ndarray:
    global LAST_EXEC_NS
    _install_ntff_hook()
    from concourse.bass_utils import run_bass_kernel_spmd

    nc = _build_program()
    in_maps = _host_prep(**inputs)
    res = run_bass_kernel_spmd(nc, in_maps, list(range(NCORES)), trace=TRACE)
    LAST_EXEC_NS = res.exec_time_ns
    out = np.concatenate([res.results[c]["out"] for c in range(NCORES)])
    return out.astype(np.float32)


# revision 4
# speedup vs baseline: 1.1894x; 1.0238x over previous
"""NeuMF (embedding lookup + tiny MLP) on 8 Trainium2 NeuronCores.

Strategy (data-parallel, per the sharding hint's "replicate tables, shard ids"):
- Host: concatenate gmf+mlp halves of each table into one combined table
  cucm[(NU+NM), 72] (user row r = [gmf_user[r] | mlp_user[r]], movie rows
  offset by NU).  Shard the 16384 batch ids 8 ways; each core gathers its
  2048 user rows + 2048 movie rows with 32 indirect DMAs (128 rows each,
  one row per SBUF partition), then computes the whole model on-chip:
    prod  = gmf_u * gmf_m                  (DVE)
    transposes via PE (batch -> free axis)
    h1    = relu(W1^T mlp_in + b1)         (block-diag matmuls, 8 t-blocks/chunk)
    h2    = relu(W2^T h1 + b2)
    logit = Wf_gmf^T prodT + Wf_mlp^T h2   (PSUM-accumulated matmuls)
    out   = 4*sigmoid(logit + bf) + 1      (ACT)
- Gather/unshard on host (pure layout permutation, no embedding math).
"""
import sys
import types
import functools

import numpy as np

# ---------------- problem constants (hardcoded per contract) ----------------
NU = 1_000_000
NM = 100_000
E = 64            # gmf embed dim
MD = 8            # mlp half dim
CW = E + MD       # combined row width (72 f32)
B = 16384
NCORES = 8
SHARD = B // NCORES   # 2048
P = 128
T = SHARD // P        # 16 t-blocks per core
GC = T // 2           # 8 gmf transpose chunks ([128,128] = 2 t-blocks)
MC = T // 8           # 2 mlp transpose chunks ([128,128] = 8 t-blocks)

TRACE = False          # test.py flips this for neuron-profile timing
LAST_EXEC_NS = None


def _install_ntff_hook():
    """bass_utils' trace path imports antenv.axon_hooks (absent here); shim it."""
    if "antenv.axon_hooks" in sys.modules:
        return
    try:
        import antenv  # noqa: F401
        mod = types.ModuleType("antenv.axon_hooks")
        mod._hook = None
        mod.set_axon_ntff_profile_hook = lambda h: setattr(mod, "_hook", h)
        mod.get_axon_ntff_profile_hook = lambda: mod._hook
        sys.modules["antenv.axon_hooks"] = mod
        from trn_agent_boot.trn_boot import _ntff_profile_via_ctypes
        mod.set_axon_ntff_profile_hook(
            _ntff_profile_via_ctypes('/opt/axon/libaxon_pjrt.so'))
    except Exception:
        pass


@functools.lru_cache(maxsize=1)
def _build_program():
    import concourse.bacc as bacc
    import concourse.bass as bass
    import concourse.tile as tile
    from concourse import mybir

    f32 = mybir.dt.float32
    i32 = mybir.dt.int32

    nc = bacc.Bacc("TRN2", target_bir_lowering=False, debug=False,
                   enable_asserts=False, num_devices=NCORES)

    # ids: [128, 32] int32; col 2t = user idx of t-block t, col 2t+1 = movie idx + NU
    ids_d = nc.dram_tensor("ids", (P, 2 * T), i32, kind="ExternalInput")
    tab_d = nc.dram_tensor("tab", (NU + NM, CW), f32, kind="ExternalInput")
    # c128: [128, 128 identity | 128 gw | 64 W1bd] = [128, 320]
    c128_d = nc.dram_tensor("c128", (P, 320), f32, kind="ExternalInput")
    # c64: [64, 32 W2bd | 32 wf4s (rows 0:32) | 1 b1r | 1 b2r | 1 bfr] = [64, 67]
    c64_d = nc.dram_tensor("c64", (64, 67), f32, kind="ExternalInput")
    out_d = nc.dram_tensor("out", (SHARD,), f32, kind="ExternalOutput")

    with tile.TileContext(nc) as tc:
        with (
            tc.tile_pool(name="const", bufs=1) as cpool,
            tc.tile_pool(name="gat", bufs=1) as gpool,
            tc.tile_pool(name="work", bufs=2) as wpool,
            tc.tile_pool(name="ps_t", bufs=2, space="PSUM") as pt_pool,
            tc.tile_pool(name="ps_m", bufs=2, space="PSUM") as pm_pool,
            tc.tile_pool(name="ps_l", bufs=1, space="PSUM") as pl_pool,
        ):
            ids = cpool.tile([P, 2 * T], i32)
            nc.sync.dma_start(out=ids[:], in_=ids_d[:])
            c128 = cpool.tile([P, 320], f32)
            nc.sync.dma_start(out=c128[:], in_=c128_d[:])
            c64 = cpool.tile([64, 67], f32)
            nc.sync.dma_start(out=c64[:], in_=c64_d[:])

            identity = c128[:, 0:128]
            gw = c128[:, 128:256]         # [128, 8 chunks x 16 cols]
            w1bd = c128[:, 256:320]       # [128, 64]
            w2bd = c64[:, 0:32]           # [64, 32]
            wf4s = c64[0:32, 32:64]       # [32, 2 chunks x 16]
            b1r = c64[:, 64:65]           # [64, 1]
            b2r = c64[0:32, 65:66]        # [32, 1]
            bfr = c64[0:16, 66:67]        # [16, 1]

            # ---- gather: 32 indirect DMAs, one [128, 72] row-block each ----
            g = gpool.tile([P, 2 * T * CW], f32)   # [128, 32, 72] flat
            for c in range(2 * T):
                nc.gpsimd.indirect_dma_start(
                    out=g[:, c * CW:(c + 1) * CW],
                    out_offset=None,
                    in_=tab_d[:],
                    in_offset=bass.IndirectOffsetOnAxis(ap=ids[:, c:c + 1], axis=0),
                )

            g3 = g[:].rearrange("p (c w) -> p c w", w=CW)   # [128, 32, 72]
            gu = g3[:, 0::2, :]    # [128, 16, 72] user rows
            gm = g3[:, 1::2, :]    # [128, 16, 72] movie rows

            from concourse.mybir import ActivationFunctionType as AFT
            # warm the sigmoid ACT LUT during the gathers, off the critical path
            warm = wpool.tile([1, 1], f32, bufs=1)
            nc.scalar.activation(out=warm[:], in_=c64[0:1, 0:1], func=AFT.Sigmoid)

            prod = wpool.tile([P, T * E], f32, bufs=1)      # [128, 16, 64]
            mlpc = wpool.tile([P, T * 16], f32, bufs=1)     # [128, 16, 16]
            m3 = mlpc[:].rearrange("p (t k) -> p t k", k=16)
            logit = pl_pool.tile([T, P], f32, space="PSUM")
            n_acc = GC + MC
            acc = 0

            def mlp_chain(c, acc, n_acc):
                mT_ps = pt_pool.tile([P, P], f32, space="PSUM", name="mT_ps", tag="tr")
                nc.tensor.transpose(
                    out=mT_ps[:], in_=mlpc[:, c * P:(c + 1) * P], identity=identity)
                mT = wpool.tile([P, P], f32, name="mT")
                nc.vector.tensor_copy(out=mT[:], in_=mT_ps[:])
                h1_ps = pm_pool.tile([64, P], f32, space="PSUM", name="h1_ps", tag="mm")
                nc.tensor.matmul(out=h1_ps[:], lhsT=w1bd, rhs=mT[:],
                                 start=True, stop=True)
                h1 = wpool.tile([64, P], f32, name="h1")
                nc.scalar.activation(out=h1[:], in_=h1_ps[:], func=AFT.Relu,
                                     bias=b1r)
                h2_ps = pm_pool.tile([64, P], f32, space="PSUM", name="h2_ps", tag="mm")
                nc.tensor.matmul(out=h2_ps[0:32, :], lhsT=w2bd, rhs=h1[:],
                                 start=True, stop=True)
                h2 = wpool.tile([32, P], f32, name="h2")
                nc.scalar.activation(out=h2[:], in_=h2_ps[0:32, :], func=AFT.Relu,
                                     bias=b2r)
                nc.tensor.matmul(
                    out=logit[:], lhsT=wf4s[:, c * 16:(c + 1) * 16], rhs=h2[:],
                    start=(acc == 0), stop=(acc == n_acc - 1))

            # ---- per 2-t-block chunk: prod/mlp prep + transpose + matmul,
            #      each starts as soon as its 4 gathers have landed ----
            for r in range(GC):
                nc.vector.tensor_mul(
                    out=prod[:, r * P:(r + 1) * P].rearrange(
                        "p (t e) -> p t e", e=E),
                    in0=gu[:, 2 * r:2 * r + 2, 0:E],
                    in1=gm[:, 2 * r:2 * r + 2, 0:E])
                nc.vector.tensor_copy(out=m3[:, 2 * r:2 * r + 2, 0:MD],
                                      in_=gu[:, 2 * r:2 * r + 2, E:CW])
                nc.vector.tensor_copy(out=m3[:, 2 * r:2 * r + 2, MD:16],
                                      in_=gm[:, 2 * r:2 * r + 2, E:CW])
                pT_ps = pt_pool.tile([P, P], f32, space="PSUM", name="pT_ps", tag="tr")
                nc.tensor.transpose(
                    out=pT_ps[:], in_=prod[:, r * P:(r + 1) * P], identity=identity)
                pT = wpool.tile([P, P], f32, name="pT")
                nc.vector.tensor_copy(out=pT[:], in_=pT_ps[:])
                nc.tensor.matmul(
                    out=logit[:], lhsT=gw[:, r * 16:(r + 1) * 16], rhs=pT[:],
                    start=(acc == 0), stop=False)
                acc += 1
                if r == 3 or r == GC - 1:
                    mlp_chain((0 if r == 3 else 1), acc, n_acc)
                    acc += 1

            # ---- sigmoid + affine + store ----
            sg = wpool.tile([T, P], f32, bufs=1)
            nc.scalar.activation(out=sg[:], in_=logit[:], func=AFT.Sigmoid,
                                 bias=bfr)
            o = wpool.tile([T, P], f32, bufs=1)
            nc.scalar.activation(out=o[:], in_=sg[:], func=AFT.Copy,
                                 scale=4.0, bias=1.0)
            nc.sync.dma_start(
                out=out_d[:].rearrange("(t p) -> t p", p=P), in_=o[:])

    nc.compile()
    return nc


def _host_prep(user_ids, movie_ids, gmf_user_emb, gmf_movie_emb,
               mlp_user_emb, mlp_movie_emb, W1, b1, W2, b2, Wf, bf):
    """Build the combined table, per-core id layouts, and constant tensors."""
    uid = np.asarray(user_ids).astype(np.int32)
    mid = np.asarray(movie_ids).astype(np.int32)
    tab = np.empty((NU + NM, CW), np.float32)
    tab[:NU, :E] = gmf_user_emb
    tab[:NU, E:] = mlp_user_emb
    tab[NU:, :E] = gmf_movie_emb
    tab[NU:, E:] = mlp_movie_emb

    W1 = np.asarray(W1, np.float32)
    W2 = np.asarray(W2, np.float32)
    Wf = np.asarray(Wf, np.float32)
    b1 = np.asarray(b1, np.float32)
    b2 = np.asarray(b2, np.float32)
    bfv = float(np.asarray(bf).reshape(-1)[0])

    identity = np.eye(P, dtype=np.float32)
    # gw: per gmf chunk r (2 t-blocks), lhsT [128, 16]:
    #   rows 0:64  (parity 0, e) -> col 2r
    #   rows 64:128 (parity 1, e) -> col 2r+1
    gw = np.zeros((P, GC * 16), np.float32)
    for r in range(GC):
        gw[0:E, r * 16 + 2 * r] = Wf[0:E, 0]
        gw[E:2 * E, r * 16 + 2 * r + 1] = Wf[0:E, 0]
    # W1 blockdiag over 8 t_l blocks: [128=(t_l,k), 64=(t_l,j)]
    w1bd = np.zeros((P, 64), np.float32)
    for tl in range(8):
        w1bd[tl * 16:(tl + 1) * 16, tl * 8:(tl + 1) * 8] = W1
    # W2 blockdiag: [64=(t_l,j), 32=(t_l,l)]
    w2bd = np.zeros((64, 32), np.float32)
    for tl in range(8):
        w2bd[tl * 8:(tl + 1) * 8, tl * 4:(tl + 1) * 4] = W2
    # wf4 stage per mlp chunk c: lhsT [32=(t_l,l), 16=t]: col 8c+t_l gets Wf[64+l]
    wf4s = np.zeros((32, MC * 16), np.float32)
    for c in range(MC):
        for tl in range(8):
            wf4s[tl * 4:(tl + 1) * 4, c * 16 + 8 * c + tl] = Wf[E:E + 4, 0]
    b1r = np.tile(b1, 8).reshape(64, 1)
    b2r = np.tile(b2, 8).reshape(32, 1)
    bfr = np.full((16, 1), bfv, np.float32)

    c128 = np.zeros((P, 320), np.float32)
    c128[:, 0:128] = identity
    c128[:, 128:256] = gw
    c128[:, 256:320] = w1bd
    c64 = np.zeros((64, 67), np.float32)
    c64[:, 0:32] = w2bd
    c64[0:32, 32:64] = wf4s
    c64[:, 64:65] = b1r
    c64[0:32, 65:66] = b2r
    c64[0:16, 66:67] = bfr

    in_maps = []
    for c in range(NCORES):
        us = uid[c * SHARD:(c + 1) * SHARD]
        ms = mid[c * SHARD:(c + 1) * SHARD] + NU
        # batch b = t*128 + p maps to ids[p, 2t] / ids[p, 2t+1]
        ids = np.empty((P, 2 * T), np.int32)
        ids[:, 0::2] = us.reshape(T, P).T
        ids[:, 1::2] = ms.reshape(T, P).T
        in_maps.append({"ids": ids, "tab": tab, "c128": c128, "c64": c64})
    return in_maps


def kernel(**inputs) -> np.ndarray:
    global LAST_EXEC_NS
    _install_ntff_hook()
    from concourse.bass_utils import run_bass_kernel_spmd

    nc = _build_program()
    in_maps = _host_prep(**inputs)
    res = run_bass_kernel_spmd(nc, in_maps, list(range(NCORES)), trace=TRACE)
    LAST_EXEC_NS = res.exec_time_ns
    out = np.concatenate([res.results[c]["out"] for c in range(NCORES)])
    return out.astype(np.float32)
